# revision 8
# baseline (speedup 1.0000x reference)
"""Distributed GQA attention kernel for one TRN2 chip (8 NeuronCores).

Sharding: tensor-parallel over heads. Core g owns query heads [4g, 4g+4)
and kv head g. Each core computes its heads' attention and a partial
output projection; a chunked ReduceScatter sums the partials and leaves
each core with a token-slice of the final output.

Layout choices (no on-device transposes of big activations):
  - x is passed pre-transposed (xT [D, B*L]) so projections contract D
    on the partition axis.
  - q/k are produced directly as qT/kT [head_dim, tokens]; scores are
    computed keys-on-partitions, so the P@V matmul consumes exp(scores)
    directly and the wo matmul consumes the attention output directly.
  - RoPE head_dim pairs are permuted (on the host, into wq/wk rows) so
    each rotation partner lives 16 partitions away within a 32-partition
    quadrant -> one DVE stream_shuffle does the swap.
  - softmax denominator comes from an all-ones matmul (partition
    broadcast for free); no max subtraction (fp32 logits here are <~15).
  - causal masking is a gpsimd affine_select zeroing exp() in the
    diagonal staircase window (no mask tensor on device at all).

Schedule (v3): the two batches are interleaved -- proj(b0), attn(b0),
proj(b1), attn(b1) -- so the ReduceScatter pipeline starts ~220us
earlier and the CC ring never backlogs into the tail. Queues: x rides
sync+gpsimd in segment 1 and the scalar HWDGE during segment 2
(prefetching b1's activations under b0's attention); wo-partial staging
rides sync; the gpsimd engine stream holds only collectives + shard
copies, so ReduceScatter can never gate a compute engine. RS outputs
land in Shared scratchpad (fast HBM-HBM path), then hop to the
external output.
"""

import numpy as np

import concourse.bass as bass
import concourse.mybir as mybir
import concourse.tile as tile
from concourse import bacc
from concourse.alu_op_type import AluOpType
from concourse.masks import make_identity

F32 = mybir.dt.float32
BF16 = mybir.dt.bfloat16

N_CORES = 8
NHL = 4           # local q heads per core
HD = 128          # head dim
THETA = 10000.0
SCALE = HD ** -0.5
TW = 512          # token block width (free dim of most matmuls)
KW = 128          # key tile width (partition dim of score tiles)
RSW = 256         # ReduceScatter chunk width (tokens per collective)

# module-level knobs for test.py
TRACE = False
LAST_RESULTS = None


class Cfg:
    def __init__(self, B=2, L=2048, D=4096):
        self.B, self.L, self.D = B, L, D
        self.BL = B * L
        self.DC = D // 128         # contraction chunks for projections
        self.NB = L // TW          # query blocks per batch
        self.NT = self.BL // TW    # token blocks total
        self.KT = L // KW          # key tiles per batch
        self.NBLK = D // TW        # wo output column blocks
        self.NCH = self.BL // RSW  # ReduceScatter chunks
        assert self.BL % TW == 0 and TW % N_CORES == 0
        assert TW % RSW == 0 and RSW % N_CORES == 0


# stream_shuffle mask: swap 16-partition halves within each 32-partition quadrant
SWAP16 = [(i + 16) % 32 for i in range(32)]


def _rope_perm():
    """Permutation of head_dim rows: pair i=(16q + r) lives at partitions
    32q+r (x1 = even dim 2i) and 32q+16+r (x2 = odd dim 2i+1)."""
    perm = np.zeros(HD, dtype=np.int64)
    for p in range(HD):
        q, r = divmod(p, 32)
        i = 16 * q + (r % 16)
        perm[p] = 2 * i + (0 if r < 16 else 1)
    return perm


def _rope_tables(cfg):
    """cosT/sinT [128, L] in the permuted-partition layout, sin sign-folded."""
    t = np.arange(cfg.L, dtype=np.float64)
    freqs = THETA ** (-np.arange(0, HD, 2, dtype=np.float64) / HD)  # [64]
    theta = t[None, :] * freqs[:, None]                             # [64, L]
    cos, sin = np.cos(theta), np.sin(theta)
    C = np.zeros((HD, cfg.L), dtype=np.float32)
    S = np.zeros((HD, cfg.L), dtype=np.float32)
    for p in range(HD):
        q, r = divmod(p, 32)
        i = 16 * q + (r % 16)
        C[p] = cos[i]
        S[p] = sin[i] if r >= 16 else -sin[i]
    return C, S


def classify_mask(mask, cfg):
    """cls[kt][qb] = (kind, off): kind in {'Z','N','M'} for tile
    mask[qb*TW:(qb+1)*TW, kt*KW:(kt+1)*KW]; off = count of leading query
    columns in the tile that are fully masked (safe to skip: exp would
    be exactly 0 there). M tiles must match the causal staircase -- the
    device applies them with an affine_select, not the mask data."""
    cls = [[None] * cfg.NB for _ in range(cfg.KT)]
    for kt in range(cfg.KT):
        for qb in range(cfg.NB):
            t = mask[qb * TW:(qb + 1) * TW, kt * KW:(kt + 1) * KW]
            if np.all(t == 0.0):
                cls[kt][qb] = ('Z', 0)
            elif np.all(t <= -1e8):
                cls[kt][qb] = ('N', 0)
            else:
                qq = np.arange(qb * TW, (qb + 1) * TW)[:, None]
                kk = np.arange(kt * KW, (kt + 1) * KW)[None, :]
                causal = kk <= qq
                assert np.all((t == 0.0) == causal) and \
                    np.all(t[~causal] <= -1e8), \
                    "partial mask tiles must be causal"
                dead_q = np.all(t <= -1e8, axis=1)  # [TW]
                off = 0
                while off < len(dead_q) and dead_q[off]:
                    off += 1
                off = (off // 64) * 64  # keep offsets 64-aligned
                cls[kt][qb] = ('M', off)
    # guard: every query block must attend to at least one key tile
    for qb in range(cfg.NB):
        assert any(cls[kt][qb][0] != 'N' for kt in range(cfg.KT)), \
            "fully-masked query block unsupported"
    return cls


def build_bass(cfg, cls):
    nc = bacc.Bacc("TRN2", target_bir_lowering=False, debug=False,
                   num_devices=N_CORES, num_swdge_queues=4)

    xT_d = nc.dram_tensor("xT", [cfg.D, cfg.BL], BF16, kind="ExternalInput")
    wqT_d = nc.dram_tensor("wqT", [cfg.D, NHL * HD], BF16, kind="ExternalInput")
    wkT_d = nc.dram_tensor("wkT", [cfg.D, HD], BF16, kind="ExternalInput")
    wvT_d = nc.dram_tensor("wvT", [cfg.D, HD], BF16, kind="ExternalInput")
    woT_d = nc.dram_tensor("woT", [NHL * HD, cfg.D], BF16, kind="ExternalInput")
    ropeC_d = nc.dram_tensor("ropeC", [HD, cfg.L], F32, kind="ExternalInput")
    ropeS_d = nc.dram_tensor("ropeS", [HD, cfg.L], F32, kind="ExternalInput")
    out_d = nc.dram_tensor("out", [cfg.BL // N_CORES, cfg.D], BF16,
                           kind="ExternalOutput")

    rg = [list(range(N_CORES))]
    QD = NHL * HD  # 512
    rsw_out = RSW // N_CORES  # output rows per RS chunk (32)

    with tile.TileContext(nc) as tc:
        # ---- constants / tables -------------------------------------------
        const_pool = tc.alloc_tile_pool(name="const", bufs=1)
        ones_sb = const_pool.tile([128, 128], BF16, name="ones_sb")
        nc.vector.memset(ones_sb[:], 1.0)
        ident = const_pool.tile([128, 128], BF16, name="ident")
        make_identity(nc, ident[:])

        # ---- resident activations -----------------------------------------
        kv_pool = tc.alloc_tile_pool(name="kv", bufs=1)
        kT_sb = kv_pool.tile([HD, cfg.BL], BF16, name="kT_sb")
        v_sb = kv_pool.tile([128, cfg.BL], BF16, name="v_sb")
        # q resident for all local heads: [hd, h*BL + tok]
        q_pool = tc.alloc_tile_pool(name="qres", bufs=1)
        qT_sb = q_pool.tile([HD, NHL * cfg.BL], BF16, name="qT_sb")

        # DRAM scratch: wo partials staged per RS chunk
        dram_pool = tc.alloc_tile_pool(name="dram", bufs=1, space="DRAM")
        rs_in = [dram_pool.tile([RSW, cfg.D], BF16, name=f"rs_in{c}")
                 for c in range(cfg.NCH)]
        rs_out = [dram_pool.tile([rsw_out, cfg.D], BF16, name=f"rs_out{c}")
                  for c in range(cfg.NCH)]

        # ---- persistent phase-1 staging -----------------------------------
        rtbl_pool = tc.alloc_tile_pool(name="ropetbl", bufs=1)
        ropeC = rtbl_pool.tile([HD, cfg.L], F32, name="ropeC_sb")
        ropeS = rtbl_pool.tile([HD, cfg.L], F32, name="ropeS_sb")
        rtmp_pool = tc.alloc_tile_pool(name="ropetmp", bufs=3)
        vst_pool = tc.alloc_tile_pool(name="vstage", bufs=2)
        x_pool = tc.alloc_tile_pool(name="xload", bufs=6)

        # ---- weights: wo persists, wq/wk/wv released after last proj ------
        wo_pool = tc.alloc_tile_pool(name="wo_w", bufs=1)
        wo_sb = wo_pool.tile([128, NHL * cfg.D], BF16, name="wo_sb")
        w_pool = tc.alloc_tile_pool(name="weights", bufs=1)
        wq_sb = w_pool.tile([128, cfg.DC * QD], BF16, name="wq_sb")
        wk_sb = w_pool.tile([128, cfg.DC * HD], BF16, name="wk_sb")
        wv_sb = w_pool.tile([128, cfg.DC * HD], BF16, name="wv_sb")

        def load_w3d(eng, dst, src_d, width, chunk, interleave=None):
            """dst[:, dc*width+c] = src[dc*128+p, c], batched `chunk` dcs/DMA.
            With interleave=(dst2, src2): alternate chunks of two tensors."""
            for d0 in range(0, cfg.DC, chunk):
                d1 = min(d0 + chunk, cfg.DC)
                for dd, ss in ((dst, src_d),) + (interleave or ()):
                    eng.dma_start(
                        out=dd[:, d0 * width:d1 * width]
                        .rearrange("p (dc c) -> p dc c", dc=d1 - d0),
                        in_=ss.ap()[d0 * 128:d1 * 128, :]
                        .rearrange("(dc p) c -> p dc c", p=128))

        # wk/wv on the SWDGE queue (gpsimd) so they don't delay x on sync;
        # interleaved so the first dc chunks of BOTH land early.
        load_w3d(nc.gpsimd, wk_sb, wkT_d, HD, 8, interleave=((wv_sb, wvT_d),))
        load_w3d(nc.scalar, wq_sb, wqT_d, QD, 4)   # 8 DMAs of 1MB (ACT queue)
        for h in range(NHL):                       # 4 DMAs of 1MB (ACT queue)
            nc.scalar.dma_start(out=wo_sb[:, h * cfg.D:(h + 1) * cfg.D],
                                in_=woT_d.ap()[h * HD:(h + 1) * HD, :])
        nc.scalar.dma_start(out=ropeC[:], in_=ropeC_d.ap())
        nc.scalar.dma_start(out=ropeS[:], in_=ropeS_d.ap())

        # ---- x loads: emitted per tokblock, possibly ahead of use ---------
        xtiles = {}  # tb -> list of per-dc [128, TW] APs
        XB = 2       # dc-chunks per DMA (1MB)

        def emit_x_loads(tb, engines):
            tiles = []
            for i, dc in enumerate(range(0, cfg.DC, XB)):
                d1 = min(dc + XB, cfg.DC)
                xt = x_pool.tile([128, (d1 - dc) * TW], BF16, name="x_t")
                engines[i % len(engines)].dma_start(
                    out=xt[:].rearrange("p (dc t) -> p dc t", dc=d1 - dc),
                    in_=xT_d.ap()[dc * 128:d1 * 128, tb * TW:(tb + 1) * TW]
                    .rearrange("(dc p) t -> p dc t", p=128))
                for j in range(d1 - dc):
                    tiles.append(xt[:, j * TW:(j + 1) * TW])
            xtiles[tb] = tiles

        # =================== per-batch segments ============================
        for b in range(cfg.B):
            # ---- projections + RoPE for batch b ---------------------------
            with tc.tile_pool(name=f"qpsum{b}", bufs=1, space="PSUM") as q_psum, \
                 tc.tile_pool(name=f"kpsum{b}", bufs=2, space="PSUM") as k_psum, \
                 tc.tile_pool(name=f"vpsum{b}", bufs=1, space="PSUM") as v_psum, \
                 tc.tile_pool(name=f"vtpsum{b}", bufs=1, space="PSUM") as vt_psum:

                def rope_drain(ps, dst):
                    """dst = ps*C + shuffle16(ps)*S (tables sliced at t0)."""
                    sw = rtmp_pool.tile([128, TW], F32, name="rope_sw")
                    t1 = rtmp_pool.tile([128, TW], F32, name="rope_t1")
                    t2 = rtmp_pool.tile([128, TW], F32, name="rope_t2")
                    nc.vector.stream_shuffle(sw[:], ps, SWAP16)
                    nc.vector.tensor_tensor(t1[:], sw[:], Sx, AluOpType.mult)
                    nc.vector.tensor_tensor(t2[:], ps, Cx, AluOpType.mult)
                    nc.vector.tensor_tensor(dst, t1[:], t2[:], AluOpType.add)

                for tbl in range(cfg.NB):
                    tb = b * cfg.NB + tbl
                    if b == 0:
                        emit_x_loads(tb, [nc.sync, nc.gpsimd])
                    t0 = tbl * TW  # position within batch
                    Cx = ropeC[:, t0:t0 + TW]
                    Sx = ropeS[:, t0:t0 + TW]

                    q_ps = q_psum.tile([128, NHL * TW], F32, name="q_ps")
                    k_ps = k_psum.tile([128, TW], F32, name="k_ps")
                    vT_ps = v_psum.tile([128, TW], F32, name="vT_ps")
                    xts = xtiles.pop(tb)
                    for dc in range(cfg.DC):
                        st = dict(start=(dc == 0), stop=(dc == cfg.DC - 1))
                        nc.tensor.matmul(k_ps[:],
                                         wk_sb[:, dc * HD:(dc + 1) * HD],
                                         xts[dc], **st)
                        nc.tensor.matmul(vT_ps[:],
                                         wv_sb[:, dc * HD:(dc + 1) * HD],
                                         xts[dc], **st)
                    for dc in range(cfg.DC):
                        st = dict(start=(dc == 0), stop=(dc == cfg.DC - 1))
                        for h in range(NHL):
                            nc.tensor.matmul(
                                q_ps[:, h * TW:h * TW + TW],
                                wq_sb[:, dc * QD + h * HD:
                                      dc * QD + (h + 1) * HD],
                                xts[dc], **st)

                    # k: rope -> resident (drain first: next tb needs bank)
                    rope_drain(k_ps[:], kT_sb[:, tb * TW:(tb + 1) * TW])
                    for h in range(NHL):
                        rope_drain(q_ps[:, h * TW:h * TW + TW],
                                   qT_sb[:, h * cfg.BL + tb * TW:
                                         h * cfg.BL + (tb + 1) * TW])
                    # v: vT -> transpose -> resident [ktok, hd] blocks
                    vt_sb = vst_pool.tile([128, TW], BF16, name="vT_stage")
                    nc.scalar.copy(vt_sb[:], vT_ps[:])
                    for i in range(TW // 128):
                        vp = vt_psum.tile([128, 128], BF16, name="v_tr_ps")
                        nc.tensor.transpose(vp[:],
                                            vt_sb[:, i * 128:(i + 1) * 128],
                                            ident[:])
                        nc.scalar.copy(
                            v_sb[:, tb * TW + i * 128:tb * TW + (i + 1) * 128],
                            vp[:])

            if b == cfg.B - 1:
                w_pool.release()  # wq/wk/wv done after the last projection

            # ---- attention + wo + ReduceScatter for batch b ---------------
            with tc.tile_pool(name=f"expsb{b}", bufs=4) as e_pool, \
                 tc.tile_pool(name=f"attnsb{b}", bufs=2) as at_pool, \
                 tc.tile_pool(name=f"recsb{b}", bufs=2) as rec_pool, \
                 tc.tile_pool(name=f"outcp{b}", bufs=3) as oc_pool, \
                 tc.tile_pool(name=f"scps{b}", bufs=3, space="PSUM") as sc_psum, \
                 tc.tile_pool(name=f"avps{b}", bufs=2, space="PSUM") as av_psum, \
                 tc.tile_pool(name=f"seps{b}", bufs=1, space="PSUM") as se_psum, \
                 tc.tile_pool(name=f"ops{b}", bufs=2, space="PSUM") as o_psum:

                for qb in range(cfg.NB):
                    if b == 0:
                        # prefetch batch 1's activations on the scalar HWDGE
                        # (its hardware queue is otherwise idle now; waits
                        # park in the queue, not on the ACT engine)
                        emit_x_loads(cfg.NB + qb, [nc.scalar])
                    active = [kt for kt in range(cfg.KT)
                              if cls[kt][qb][0] != 'N']
                    offs = {kt: cls[kt][qb][1] for kt in active}
                    offs[active[0]] = 0

                    attn_sb = at_pool.tile([128, NHL * TW], BF16, name="at_sb")
                    tb2 = b * cfg.NB + qb
                    for h in range(NHL):
                        qt = qT_sb[:, h * cfg.BL + tb2 * TW:
                                   h * cfg.BL + (tb2 + 1) * TW]
                        at_ps = av_psum.tile([HD, TW], F32, name="at_ps")
                        se_ps = se_psum.tile([128, TW], F32, name="se_ps")
                        # software pipeline: issue score matmuls LOOKAHEAD
                        # iterations ahead so the PE never waits on exp (ACT)
                        LOOKAHEAD = 2
                        n_act = len(active)
                        sc_tiles = [None] * n_act

                        def emit_sc(j):
                            kt2 = active[j]
                            gk2 = b * cfg.L + kt2 * KW
                            o = offs[kt2]
                            sc = sc_psum.tile([KW, TW], F32, name="sc_ps")
                            nc.tensor.matmul(sc[:, o:], kT_sb[:, gk2:gk2 + KW],
                                             qt[:, o:], start=True, stop=True)
                            sc_tiles[j] = sc

                        for j in range(min(LOOKAHEAD, n_act)):
                            emit_sc(j)
                        for idx, kt in enumerate(active):
                            if idx + LOOKAHEAD < n_act:
                                emit_sc(idx + LOOKAHEAD)
                            gk = b * cfg.L + kt * KW  # global key token
                            o = offs[kt]
                            sc_ps = sc_tiles[idx]
                            sc_tiles[idx] = None
                            ex = e_pool.tile([KW, TW], BF16, name="ex_t")
                            nc.scalar.activation(
                                ex[:, o:], sc_ps[:, o:],
                                mybir.ActivationFunctionType.Exp,
                                scale=float(SCALE))
                            if cls[kt][qb][0] == 'M':
                                # zero the masked staircase: keep key p on
                                # query column j iff qb*TW+o+j >= kt*KW+p.
                                # Columns past wend see all 128 keys.
                                wend = kt * KW + KW - qb * TW
                                nc.gpsimd.affine_select(
                                    out=ex[:, o:wend], in_=ex[:, o:wend],
                                    pattern=[[1, wend - o]],
                                    compare_op=AluOpType.is_ge,
                                    fill=0.0,
                                    base=qb * TW + o - kt * KW,
                                    channel_multiplier=-1)
                            st = dict(start=(idx == 0),
                                      stop=(idx == len(active) - 1))
                            nc.tensor.matmul(se_ps[:, o:], ones_sb[:],
                                             ex[:, o:], **st)
                            nc.tensor.matmul(at_ps[:, o:], v_sb[:, gk:gk + KW],
                                             ex[:, o:], **st)
                        rec = rec_pool.tile([128, TW], F32, name="rec_t")
                        nc.vector.reciprocal_approx_fast(rec[:], se_ps[:])
                        nc.vector.tensor_tensor(
                            attn_sb[:, h * TW:(h + 1) * TW],
                            at_ps[:], rec[:], AluOpType.mult)

                    # ---- wo partial for this (b, qb) token block ----------
                    c = b * cfg.NB + qb
                    NG = min(4, cfg.NBLK)  # n-blocks per batched store
                    for m in range(TW // 128):
                        c2 = c * (TW // RSW) + (m * 128) // RSW  # RS chunk
                        mr = (m * 128) % RSW                     # row in chunk
                        for ng in range(cfg.NBLK // NG):
                            oc = oc_pool.tile([128, NG * TW], BF16,
                                              name="oc_t")
                            for j in range(NG):
                                n = ng * NG + j
                                o_ps = o_psum.tile([128, TW], F32,
                                                   name="o_ps")
                                for h in range(NHL):
                                    nc.tensor.matmul(
                                        o_ps[:],
                                        attn_sb[:, h * TW + m * 128:
                                                h * TW + (m + 1) * 128],
                                        wo_sb[:, h * cfg.D + n * TW:
                                              h * cfg.D + (n + 1) * TW],
                                        start=(h == 0), stop=(h == NHL - 1))
                                # alternate drain engine: keep ACT free
                                dst = oc[:, j * TW:(j + 1) * TW]
                                if n % 2 == 0:
                                    nc.vector.tensor_copy(dst, o_ps[:])
                                else:
                                    nc.scalar.copy(dst, o_ps[:])
                            # stage to DRAM on the sync HWDGE queue: the
                            # collective's queue must never gate compute
                            nc.sync.dma_start(
                                out=rs_in[c2][mr:mr + 128,
                                              ng * NG * TW:(ng + 1) * NG * TW],
                                in_=oc[:])
                    # ReduceScatter each finished half-block; the gpsimd
                    # stream holds only collectives + tiny shard copies
                    for s in range(TW // RSW):
                        c2 = c * (TW // RSW) + s
                        nc.gpsimd.collective_compute(
                            "ReduceScatter", AluOpType.add, replica_groups=rg,
                            ins=[rs_in[c2][:].opt()],
                            outs=[rs_out[c2][:].opt()])
                        nc.gpsimd.dma_start(
                            out=out_d.ap()[c2 * rsw_out:(c2 + 1) * rsw_out, :],
                            in_=rs_out[c2][:])

        wo_pool.release()
        x_pool.release()
        vst_pool.release()
        rtmp_pool.release()
        rtbl_pool.release()
        dram_pool.release()
        q_pool.release()
        kv_pool.release()
        const_pool.release()

    nc.compile()
    return nc


def host_prepare(cfg, x, mask, wq, wk, wv, wo):
    """Returns (in_maps, cls)."""
    x = np.ascontiguousarray(np.asarray(x, dtype=np.float32))
    mask = np.asarray(mask, dtype=np.float32)
    wq = np.asarray(wq, dtype=np.float32)
    wk = np.asarray(wk, dtype=np.float32)
    wv = np.asarray(wv, dtype=np.float32)
    wo = np.asarray(wo, dtype=np.float32)

    import ml_dtypes
    bf16 = ml_dtypes.bfloat16
    perm = _rope_perm()
    C, S = _rope_tables(cfg)
    xT = np.ascontiguousarray(x.reshape(cfg.BL, cfg.D).T).astype(bf16)
    cls = classify_mask(mask, cfg)

    in_maps = []
    for g in range(N_CORES):
        qrows = wq[g * NHL * HD:(g + 1) * NHL * HD]          # [512, D]
        qperm = np.concatenate(
            [qrows[h * HD + perm] for h in range(NHL)], axis=0)
        krows = wk[g * HD:(g + 1) * HD][perm]                # [128, D]
        vrows = wv[g * HD:(g + 1) * HD]                      # [128, D]
        wocols = wo[:, g * NHL * HD:(g + 1) * NHL * HD]      # [D, 512]
        in_maps.append({
            "xT": xT,
            "wqT": np.ascontiguousarray(qperm.T).astype(bf16),
            "wkT": np.ascontiguousarray(krows.T).astype(bf16),
            "wvT": np.ascontiguousarray(vrows.T).astype(bf16),
            "woT": np.ascontiguousarray(wocols.T).astype(bf16),
            "ropeC": C,
            "ropeS": S,
        })
    return in_maps, cls


def assemble_output(cfg, results):
    """Stitch per-core ReduceScatter shards back into [B, L, D]."""
    full = np.empty((cfg.BL, cfg.D), dtype=np.float32)
    rw = RSW // N_CORES
    for g in range(N_CORES):
        r = np.asarray(results[g]["out"]).astype(np.float32)
        for c in range(cfg.NCH):
            full[c * RSW + g * rw: c * RSW + (g + 1) * rw] = \
                r[c * rw:(c + 1) * rw]
    return full.reshape(cfg.B, cfg.L, cfg.D)


def kernel(x, mask, wq, wk, wv, wo):
    global LAST_RESULTS
    from concourse.bass_utils import run_bass_kernel_spmd
    cfg = Cfg(B=2, L=2048, D=4096)
    in_maps, cls = host_prepare(cfg, x, mask, wq, wk, wv, wo)
    nc = build_bass(cfg, cls)
    res = run_bass_kernel_spmd(nc, in_maps, core_ids=list(range(N_CORES)),
                               trace=TRACE)
    LAST_RESULTS = res
    return assemble_output(cfg, res.results)


# revision 9
# speedup vs baseline: 1.1820x; 1.1820x over previous
"""Distributed GQA attention kernel for one TRN2 chip (8 NeuronCores).

Sharding: tensor-parallel over heads. Core g owns query heads [4g, 4g+4)
and kv head g. Each core computes its heads' attention and a partial
output projection; a chunked ReduceScatter sums the partials and leaves
each core with a token-slice of the final output.

Layout choices (no on-device transposes of big activations):
  - x is passed pre-transposed (xT [D, B*L]) so projections contract D
    on the partition axis.
  - q/k are produced directly as qT/kT [head_dim, tokens]; scores are
    computed keys-on-partitions, so the P@V matmul consumes exp(scores)
    directly and the wo matmul consumes the attention output directly.
  - RoPE head_dim pairs are permuted (on the host, into wq/wk rows) so
    each rotation partner lives 16 partitions away within a 32-partition
    quadrant -> one DVE stream_shuffle does the swap.
  - softmax denominator comes from an all-ones matmul (partition
    broadcast for free); no max subtraction (fp32 logits here are <~15).
  - causal masking is a gpsimd affine_select zeroing exp() in the
    diagonal staircase window (no mask tensor on device at all).

Schedule (v3): the two batches are interleaved -- proj(b0), attn(b0),
proj(b1), attn(b1) -- so the ReduceScatter pipeline starts ~220us
earlier and the CC ring never backlogs into the tail. Queues: x rides
sync+gpsimd in segment 1 and the scalar HWDGE during segment 2
(prefetching b1's activations under b0's attention); wo-partial staging
rides sync; the gpsimd engine stream holds only collectives + shard
copies, so ReduceScatter can never gate a compute engine. RS outputs
land in Shared scratchpad (fast HBM-HBM path), then hop to the
external output.
"""

import numpy as np

import concourse.bass as bass
import concourse.mybir as mybir
import concourse.tile as tile
from concourse import bacc
from concourse.alu_op_type import AluOpType
from concourse.masks import make_identity, make_upper_triangular

F32 = mybir.dt.float32
BF16 = mybir.dt.bfloat16

N_CORES = 8
NHL = 4           # local q heads per core
HD = 128          # head dim
THETA = 10000.0
SCALE = HD ** -0.5
TW = 512          # token block width (free dim of most matmuls)
KW = 128          # key tile width (partition dim of score tiles)
RSW = 256         # ReduceScatter chunk width (tokens per collective)

# module-level knobs for test.py
TRACE = False
LAST_RESULTS = None


class Cfg:
    def __init__(self, B=2, L=2048, D=4096):
        self.B, self.L, self.D = B, L, D
        self.BL = B * L
        self.DC = D // 128         # contraction chunks for projections
        self.NB = L // TW          # query blocks per batch
        self.NT = self.BL // TW    # token blocks total
        self.KT = L // KW          # key tiles per batch
        self.NBLK = D // TW        # wo output column blocks
        self.NCH = self.BL // RSW  # ReduceScatter chunks
        assert self.BL % TW == 0 and TW % N_CORES == 0
        assert TW % RSW == 0 and RSW % N_CORES == 0


# stream_shuffle mask: swap 16-partition halves within each 32-partition quadrant
SWAP16 = [(i + 16) % 32 for i in range(32)]


def _rope_perm():
    """Permutation of head_dim rows: pair i=(16q + r) lives at partitions
    32q+r (x1 = even dim 2i) and 32q+16+r (x2 = odd dim 2i+1)."""
    perm = np.zeros(HD, dtype=np.int64)
    for p in range(HD):
        q, r = divmod(p, 32)
        i = 16 * q + (r % 16)
        perm[p] = 2 * i + (0 if r < 16 else 1)
    return perm


def _rope_tables(cfg):
    """cosT/sinT [128, L] in the permuted-partition layout, sin sign-folded."""
    t = np.arange(cfg.L, dtype=np.float64)
    freqs = THETA ** (-np.arange(0, HD, 2, dtype=np.float64) / HD)  # [64]
    theta = t[None, :] * freqs[:, None]                             # [64, L]
    cos, sin = np.cos(theta), np.sin(theta)
    C = np.zeros((HD, cfg.L), dtype=np.float32)
    S = np.zeros((HD, cfg.L), dtype=np.float32)
    for p in range(HD):
        q, r = divmod(p, 32)
        i = 16 * q + (r % 16)
        C[p] = cos[i]
        S[p] = sin[i] if r >= 16 else -sin[i]
    return C, S


def classify_mask(mask, cfg):
    """cls[kt][qb] = (kind, off): kind in {'Z','N','M'} for tile
    mask[qb*TW:(qb+1)*TW, kt*KW:(kt+1)*KW]; off = count of leading query
    columns in the tile that are fully masked (safe to skip: exp would
    be exactly 0 there). M tiles must match the causal staircase -- the
    device applies them with an affine_select, not the mask data."""
    cls = [[None] * cfg.NB for _ in range(cfg.KT)]
    for kt in range(cfg.KT):
        for qb in range(cfg.NB):
            t = mask[qb * TW:(qb + 1) * TW, kt * KW:(kt + 1) * KW]
            if np.all(t == 0.0):
                cls[kt][qb] = ('Z', 0)
            elif np.all(t <= -1e8):
                cls[kt][qb] = ('N', 0)
            else:
                qq = np.arange(qb * TW, (qb + 1) * TW)[:, None]
                kk = np.arange(kt * KW, (kt + 1) * KW)[None, :]
                causal = kk <= qq
                assert np.all((t == 0.0) == causal) and \
                    np.all(t[~causal] <= -1e8), \
                    "partial mask tiles must be causal"
                dead_q = np.all(t <= -1e8, axis=1)  # [TW]
                off = 0
                while off < len(dead_q) and dead_q[off]:
                    off += 1
                off = (off // 64) * 64  # keep offsets 64-aligned
                cls[kt][qb] = ('M', off)
    # guard: every query block must attend to at least one key tile
    for qb in range(cfg.NB):
        assert any(cls[kt][qb][0] != 'N' for kt in range(cfg.KT)), \
            "fully-masked query block unsupported"
    return cls


def build_bass(cfg, cls):
    nc = bacc.Bacc("TRN2", target_bir_lowering=False, debug=False,
                   num_devices=N_CORES, num_swdge_queues=4)

    xT_d = nc.dram_tensor("xT", [cfg.D, cfg.BL], BF16, kind="ExternalInput")
    wqT_d = nc.dram_tensor("wqT", [cfg.D, NHL * HD], BF16, kind="ExternalInput")
    wkT_d = nc.dram_tensor("wkT", [cfg.D, HD], BF16, kind="ExternalInput")
    wvT_d = nc.dram_tensor("wvT", [cfg.D, HD], BF16, kind="ExternalInput")
    woT_d = nc.dram_tensor("woT", [NHL * HD, cfg.D], BF16, kind="ExternalInput")
    ropeC_d = nc.dram_tensor("ropeC", [HD, cfg.L], F32, kind="ExternalInput")
    ropeS_d = nc.dram_tensor("ropeS", [HD, cfg.L], F32, kind="ExternalInput")
    out_d = nc.dram_tensor("out", [cfg.BL // N_CORES, cfg.D], BF16,
                           kind="ExternalOutput")

    rg = [list(range(N_CORES))]
    QD = NHL * HD  # 512
    rsw_out = RSW // N_CORES  # output rows per RS chunk (32)

    with tile.TileContext(nc) as tc:
        # ---- constants / tables -------------------------------------------
        const_pool = tc.alloc_tile_pool(name="const", bufs=1)
        ones_sb = const_pool.tile([128, 128], BF16, name="ones_sb")
        nc.vector.memset(ones_sb[:], 1.0)
        ident = const_pool.tile([128, 128], BF16, name="ident")
        make_identity(nc, ident[:])
        # causal staircase: tri[p, u] = 1 iff u >= p (keep key p on query u)
        tri_sb = const_pool.tile([128, 128], BF16, name="tri_sb")
        make_upper_triangular(nc, tri_sb[:], val=1.0, diag=True)

        # ---- resident activations -----------------------------------------
        kv_pool = tc.alloc_tile_pool(name="kv", bufs=1)
        kT_sb = kv_pool.tile([HD, cfg.BL], BF16, name="kT_sb")
        v_sb = kv_pool.tile([128, cfg.BL], BF16, name="v_sb")
        # q resident for all local heads: [hd, h*BL + tok]
        q_pool = tc.alloc_tile_pool(name="qres", bufs=1)
        qT_sb = q_pool.tile([HD, NHL * cfg.BL], BF16, name="qT_sb")

        # DRAM scratch: wo partials staged per RS chunk
        dram_pool = tc.alloc_tile_pool(name="dram", bufs=1, space="DRAM")
        rs_in = [dram_pool.tile([RSW, cfg.D], BF16, name=f"rs_in{c}")
                 for c in range(cfg.NCH)]
        rs_out = [dram_pool.tile([rsw_out, cfg.D], BF16, name=f"rs_out{c}")
                  for c in range(cfg.NCH)]

        # ---- persistent phase-1 staging -----------------------------------
        rtbl_pool = tc.alloc_tile_pool(name="ropetbl", bufs=1)
        ropeC = rtbl_pool.tile([HD, cfg.L], F32, name="ropeC_sb")
        ropeS = rtbl_pool.tile([HD, cfg.L], F32, name="ropeS_sb")
        rtmp_pool = tc.alloc_tile_pool(name="ropetmp", bufs=3)
        vst_pool = tc.alloc_tile_pool(name="vstage", bufs=2)
        x_pool = tc.alloc_tile_pool(name="xload", bufs=7)

        # ---- weights: wo persists, wq/wk/wv released after last proj ------
        wo_pool = tc.alloc_tile_pool(name="wo_w", bufs=1)
        wo_sb = wo_pool.tile([128, NHL * cfg.D], BF16, name="wo_sb")
        w_pool = tc.alloc_tile_pool(name="weights", bufs=1)
        wq_sb = w_pool.tile([128, cfg.DC * QD], BF16, name="wq_sb")
        wk_sb = w_pool.tile([128, cfg.DC * HD], BF16, name="wk_sb")
        wv_sb = w_pool.tile([128, cfg.DC * HD], BF16, name="wv_sb")

        def load_w3d(eng, dst, src_d, width, chunk, interleave=None):
            """dst[:, dc*width+c] = src[dc*128+p, c], batched `chunk` dcs/DMA.
            With interleave=(dst2, src2): alternate chunks of two tensors."""
            for d0 in range(0, cfg.DC, chunk):
                d1 = min(d0 + chunk, cfg.DC)
                for dd, ss in ((dst, src_d),) + (interleave or ()):
                    eng.dma_start(
                        out=dd[:, d0 * width:d1 * width]
                        .rearrange("p (dc c) -> p dc c", dc=d1 - d0),
                        in_=ss.ap()[d0 * 128:d1 * 128, :]
                        .rearrange("(dc p) c -> p dc c", p=128))

        # wk/wv on the SWDGE queue (gpsimd) so they don't delay x on sync;
        # interleaved so the first dc chunks of BOTH land early.
        load_w3d(nc.gpsimd, wk_sb, wkT_d, HD, 8, interleave=((wv_sb, wvT_d),))
        load_w3d(nc.scalar, wq_sb, wqT_d, QD, 4)   # 8 DMAs of 1MB (ACT queue)
        for h in range(NHL):                       # 4 DMAs of 1MB (ACT queue)
            nc.scalar.dma_start(out=wo_sb[:, h * cfg.D:(h + 1) * cfg.D],
                                in_=woT_d.ap()[h * HD:(h + 1) * HD, :])
        nc.scalar.dma_start(out=ropeC[:], in_=ropeC_d.ap())
        nc.scalar.dma_start(out=ropeS[:], in_=ropeS_d.ap())

        # ---- x loads: emitted per tokblock, possibly ahead of use ---------
        xtiles = {}  # tb -> list of per-dc [128, TW] APs
        XB = 2       # dc-chunks per DMA (1MB)

        def emit_x_loads(tb, engines):
            tiles = []
            for i, dc in enumerate(range(0, cfg.DC, XB)):
                d1 = min(dc + XB, cfg.DC)
                xt = x_pool.tile([128, (d1 - dc) * TW], BF16, name="x_t")
                engines[i % len(engines)].dma_start(
                    out=xt[:].rearrange("p (dc t) -> p dc t", dc=d1 - dc),
                    in_=xT_d.ap()[dc * 128:d1 * 128, tb * TW:(tb + 1) * TW]
                    .rearrange("(dc p) t -> p dc t", p=128))
                for j in range(d1 - dc):
                    tiles.append(xt[:, j * TW:(j + 1) * TW])
            xtiles[tb] = tiles

        # =================== per-batch segments ============================
        for b in range(cfg.B):
            # ---- projections + RoPE for batch b ---------------------------
            with tc.tile_pool(name=f"qpsum{b}", bufs=1, space="PSUM") as q_psum, \
                 tc.tile_pool(name=f"kpsum{b}", bufs=2, space="PSUM") as k_psum, \
                 tc.tile_pool(name=f"vpsum{b}", bufs=1, space="PSUM") as v_psum, \
                 tc.tile_pool(name=f"vtpsum{b}", bufs=1, space="PSUM") as vt_psum:

                def rope_drain(ps, dst):
                    """dst = ps*C + shuffle16(ps)*S (tables sliced at t0)."""
                    sw = rtmp_pool.tile([128, TW], F32, name="rope_sw")
                    t1 = rtmp_pool.tile([128, TW], F32, name="rope_t1")
                    t2 = rtmp_pool.tile([128, TW], F32, name="rope_t2")
                    nc.vector.stream_shuffle(sw[:], ps, SWAP16)
                    nc.vector.tensor_tensor(t1[:], sw[:], Sx, AluOpType.mult)
                    nc.vector.tensor_tensor(t2[:], ps, Cx, AluOpType.mult)
                    nc.vector.tensor_tensor(dst, t1[:], t2[:], AluOpType.add)

                for tbl in range(cfg.NB):
                    tb = b * cfg.NB + tbl
                    if b == 0:
                        emit_x_loads(tb, [nc.sync, nc.gpsimd])
                    t0 = tbl * TW  # position within batch
                    Cx = ropeC[:, t0:t0 + TW]
                    Sx = ropeS[:, t0:t0 + TW]

                    q_ps = q_psum.tile([128, NHL * TW], F32, name="q_ps")
                    k_ps = k_psum.tile([128, TW], F32, name="k_ps")
                    vT_ps = v_psum.tile([128, TW], F32, name="vT_ps")
                    xts = xtiles.pop(tb)
                    for dc in range(cfg.DC):
                        st = dict(start=(dc == 0), stop=(dc == cfg.DC - 1))
                        nc.tensor.matmul(k_ps[:],
                                         wk_sb[:, dc * HD:(dc + 1) * HD],
                                         xts[dc], **st)
                        nc.tensor.matmul(vT_ps[:],
                                         wv_sb[:, dc * HD:(dc + 1) * HD],
                                         xts[dc], **st)
                    for dc in range(cfg.DC):
                        st = dict(start=(dc == 0), stop=(dc == cfg.DC - 1))
                        for h in range(NHL):
                            nc.tensor.matmul(
                                q_ps[:, h * TW:h * TW + TW],
                                wq_sb[:, dc * QD + h * HD:
                                      dc * QD + (h + 1) * HD],
                                xts[dc], **st)

                    # k: rope -> resident (drain first: next tb needs bank)
                    rope_drain(k_ps[:], kT_sb[:, tb * TW:(tb + 1) * TW])
                    for h in range(NHL):
                        rope_drain(q_ps[:, h * TW:h * TW + TW],
                                   qT_sb[:, h * cfg.BL + tb * TW:
                                         h * cfg.BL + (tb + 1) * TW])
                    # v: vT -> transpose -> resident [ktok, hd] blocks
                    vt_sb = vst_pool.tile([128, TW], BF16, name="vT_stage")
                    nc.scalar.copy(vt_sb[:], vT_ps[:])
                    for i in range(TW // 128):
                        vp = vt_psum.tile([128, 128], BF16, name="v_tr_ps")
                        nc.tensor.transpose(vp[:],
                                            vt_sb[:, i * 128:(i + 1) * 128],
                                            ident[:])
                        nc.scalar.copy(
                            v_sb[:, tb * TW + i * 128:tb * TW + (i + 1) * 128],
                            vp[:])

            if b == cfg.B - 1:
                w_pool.release()  # wq/wk/wv done after the last projection

            # ---- attention + wo + ReduceScatter for batch b ---------------
            with tc.tile_pool(name=f"expsb{b}", bufs=3) as e_pool, \
                 tc.tile_pool(name=f"attnsb{b}", bufs=2) as at_pool, \
                 tc.tile_pool(name=f"recsb{b}", bufs=2) as rec_pool, \
                 tc.tile_pool(name=f"outcp{b}", bufs=3) as oc_pool, \
                 tc.tile_pool(name=f"scps{b}", bufs=3, space="PSUM") as sc_psum, \
                 tc.tile_pool(name=f"avps{b}", bufs=2, space="PSUM") as av_psum, \
                 tc.tile_pool(name=f"seps{b}", bufs=1, space="PSUM") as se_psum, \
                 tc.tile_pool(name=f"ops{b}", bufs=2, space="PSUM") as o_psum:

                for qb in range(cfg.NB):
                    if b == 0:
                        # prefetch batch 1's activations on the scalar HWDGE
                        # (its hardware queue is otherwise idle now; waits
                        # park in the queue, not on the ACT engine)
                        emit_x_loads(cfg.NB + qb, [nc.scalar])
                    active = [kt for kt in range(cfg.KT)
                              if cls[kt][qb][0] != 'N']
                    offs = {kt: cls[kt][qb][1] for kt in active}
                    offs[active[0]] = 0

                    attn_sb = at_pool.tile([128, NHL * TW], BF16, name="at_sb")
                    tb2 = b * cfg.NB + qb
                    for h in range(NHL):
                        qt = qT_sb[:, h * cfg.BL + tb2 * TW:
                                   h * cfg.BL + (tb2 + 1) * TW]
                        at_ps = av_psum.tile([HD, TW], F32, name="at_ps")
                        se_ps = se_psum.tile([128, TW], F32, name="se_ps")
                        # software pipeline: issue score matmuls LOOKAHEAD
                        # iterations ahead so the PE never waits on exp (ACT)
                        LOOKAHEAD = 2
                        n_act = len(active)
                        sc_tiles = [None] * n_act

                        def emit_sc(j):
                            kt2 = active[j]
                            gk2 = b * cfg.L + kt2 * KW
                            o = offs[kt2]
                            sc = sc_psum.tile([KW, TW], F32, name="sc_ps")
                            nc.tensor.matmul(sc[:, o:], kT_sb[:, gk2:gk2 + KW],
                                             qt[:, o:], start=True, stop=True)
                            sc_tiles[j] = sc

                        for j in range(min(LOOKAHEAD, n_act)):
                            emit_sc(j)
                        for idx, kt in enumerate(active):
                            if idx + LOOKAHEAD < n_act:
                                emit_sc(idx + LOOKAHEAD)
                            gk = b * cfg.L + kt * KW  # global key token
                            o = offs[kt]
                            sc_ps = sc_tiles[idx]
                            sc_tiles[idx] = None
                            ex = e_pool.tile([KW, TW], BF16, name="ex_t")
                            nc.scalar.activation(
                                ex[:, o:], sc_ps[:, o:],
                                mybir.ActivationFunctionType.Exp,
                                scale=float(SCALE))
                            if cls[kt][qb][0] == 'M':
                                # zero the masked staircase, which for
                                # 128-aligned tiles is exactly the 128
                                # query columns starting at w0: key p is
                                # kept on window column u iff u >= p.
                                w0 = kt * KW - qb * TW
                                nc.vector.tensor_tensor(
                                    ex[:, w0:w0 + KW], ex[:, w0:w0 + KW],
                                    tri_sb[:], AluOpType.mult)
                            st = dict(start=(idx == 0),
                                      stop=(idx == len(active) - 1))
                            nc.tensor.matmul(se_ps[:, o:], ones_sb[:],
                                             ex[:, o:], **st)
                            nc.tensor.matmul(at_ps[:, o:], v_sb[:, gk:gk + KW],
                                             ex[:, o:], **st)
                        rec = rec_pool.tile([128, TW], F32, name="rec_t")
                        nc.vector.reciprocal_approx_fast(rec[:], se_ps[:])
                        nc.vector.tensor_tensor(
                            attn_sb[:, h * TW:(h + 1) * TW],
                            at_ps[:], rec[:], AluOpType.mult)

                    # ---- wo partial for this (b, qb) token block ----------
                    c = b * cfg.NB + qb
                    NG = min(4, cfg.NBLK)  # n-blocks per batched store
                    for m in range(TW // 128):
                        c2 = c * (TW // RSW) + (m * 128) // RSW  # RS chunk
                        mr = (m * 128) % RSW                     # row in chunk
                        for ng in range(cfg.NBLK // NG):
                            oc = oc_pool.tile([128, NG * TW], BF16,
                                              name="oc_t")
                            for j in range(NG):
                                n = ng * NG + j
                                o_ps = o_psum.tile([128, TW], F32,
                                                   name="o_ps")
                                for h in range(NHL):
                                    nc.tensor.matmul(
                                        o_ps[:],
                                        attn_sb[:, h * TW + m * 128:
                                                h * TW + (m + 1) * 128],
                                        wo_sb[:, h * cfg.D + n * TW:
                                              h * cfg.D + (n + 1) * TW],
                                        start=(h == 0), stop=(h == NHL - 1))
                                # alternate drain engine: keep ACT free
                                dst = oc[:, j * TW:(j + 1) * TW]
                                if n % 2 == 0:
                                    nc.vector.tensor_copy(dst, o_ps[:])
                                else:
                                    nc.scalar.copy(dst, o_ps[:])
                            # stage to DRAM on the sync HWDGE queue: the
                            # collective's queue must never gate compute
                            nc.sync.dma_start(
                                out=rs_in[c2][mr:mr + 128,
                                              ng * NG * TW:(ng + 1) * NG * TW],
                                in_=oc[:])
                    # ReduceScatter each finished half-block; the gpsimd
                    # stream holds only collectives + tiny shard copies
                    for s in range(TW // RSW):
                        c2 = c * (TW // RSW) + s
                        nc.gpsimd.collective_compute(
                            "ReduceScatter", AluOpType.add, replica_groups=rg,
                            ins=[rs_in[c2][:].opt()],
                            outs=[rs_out[c2][:].opt()])
                        cp_eng = nc.sync if c2 == cfg.NCH - 1 else nc.gpsimd
                        cp_eng.dma_start(
                            out=out_d.ap()[c2 * rsw_out:(c2 + 1) * rsw_out, :],
                            in_=rs_out[c2][:])

        wo_pool.release()
        x_pool.release()
        vst_pool.release()
        rtmp_pool.release()
        rtbl_pool.release()
        dram_pool.release()
        q_pool.release()
        kv_pool.release()
        const_pool.release()

    nc.compile()
    return nc


def host_prepare(cfg, x, mask, wq, wk, wv, wo):
    """Returns (in_maps, cls)."""
    x = np.ascontiguousarray(np.asarray(x, dtype=np.float32))
    mask = np.asarray(mask, dtype=np.float32)
    wq = np.asarray(wq, dtype=np.float32)
    wk = np.asarray(wk, dtype=np.float32)
    wv = np.asarray(wv, dtype=np.float32)
    wo = np.asarray(wo, dtype=np.float32)

    import ml_dtypes
    bf16 = ml_dtypes.bfloat16
    perm = _rope_perm()
    C, S = _rope_tables(cfg)
    xT = np.ascontiguousarray(x.reshape(cfg.BL, cfg.D).T).astype(bf16)
    cls = classify_mask(mask, cfg)

    in_maps = []
    for g in range(N_CORES):
        qrows = wq[g * NHL * HD:(g + 1) * NHL * HD]          # [512, D]
        qperm = np.concatenate(
            [qrows[h * HD + perm] for h in range(NHL)], axis=0)
        krows = wk[g * HD:(g + 1) * HD][perm]                # [128, D]
        vrows = wv[g * HD:(g + 1) * HD]                      # [128, D]
        wocols = wo[:, g * NHL * HD:(g + 1) * NHL * HD]      # [D, 512]
        in_maps.append({
            "xT": xT,
            "wqT": np.ascontiguousarray(qperm.T).astype(bf16),
            "wkT": np.ascontiguousarray(krows.T).astype(bf16),
            "wvT": np.ascontiguousarray(vrows.T).astype(bf16),
            "woT": np.ascontiguousarray(wocols.T).astype(bf16),
            "ropeC": C,
            "ropeS": S,
        })
    return in_maps, cls


def assemble_output(cfg, results):
    """Stitch per-core ReduceScatter shards back into [B, L, D]."""
    full = np.empty((cfg.BL, cfg.D), dtype=np.float32)
    rw = RSW // N_CORES
    for g in range(N_CORES):
        r = np.asarray(results[g]["out"]).astype(np.float32)
        for c in range(cfg.NCH):
            full[c * RSW + g * rw: c * RSW + (g + 1) * rw] = \
                r[c * rw:(c + 1) * rw]
    return full.reshape(cfg.B, cfg.L, cfg.D)


def kernel(x, mask, wq, wk, wv, wo):
    global LAST_RESULTS
    from concourse.bass_utils import run_bass_kernel_spmd
    cfg = Cfg(B=2, L=2048, D=4096)
    in_maps, cls = host_prepare(cfg, x, mask, wq, wk, wv, wo)
    nc = build_bass(cfg, cls)
    res = run_bass_kernel_spmd(nc, in_maps, core_ids=list(range(N_CORES)),
                               trace=TRACE)
    LAST_RESULTS = res
    return assemble_output(cfg, res.results)


# revision 10
# speedup vs baseline: 1.4306x; 1.2102x over previous
"""Distributed GQA attention kernel for one TRN2 chip (8 NeuronCores).

Sharding: tensor-parallel over heads. Core g owns query heads [4g, 4g+4)
and kv head g. Each core computes its heads' attention and a partial
output projection; a chunked ReduceScatter sums the partials and leaves
each core with a token-slice of the final output.

Layout choices (no on-device transposes of big activations):
  - x is passed pre-transposed (xT [D, B*L]) so projections contract D
    on the partition axis.
  - q/k are produced directly as qT/kT [head_dim, tokens] and stay
    resident in SBUF; scores are computed keys-on-partitions, so the
    P@V matmul consumes exp(scores) directly and the wo matmul consumes
    the attention output directly.
  - RoPE head_dim pairs are permuted (on the host, into wq/wk rows) so
    each rotation partner lives 16 partitions away within a 32-partition
    quadrant -> one DVE stream_shuffle does the swap.
  - causal masking multiplies exp() by a constant 128x128 triangular
    0/1 tile on the DVE (the staircase window of a diagonal tile is
    always exactly 128 query columns); no mask tensor on device.
  - softmax denominator: exp tiles are accumulated on the DVE (bf16,
    2x mode) and a single all-ones matmul per head turns the sum into
    a partition-replicated PSUM tile (broadcast for free); no max
    subtraction (fp32 logits here are <~15).

Queue layout: x loads own the sync HWDGE in phase 1 and wo-partial
staging owns it in phase 2; wk/wv ride the gpsimd SWDGE and wq/wo/rope
the scalar HWDGE at startup so the first matmul fires ~10us in. The
gpsimd engine stream holds only collectives + output shard copies, so
ReduceScatter can never gate a compute engine. The last block's
ReduceScatter is split in half so the tail only waits on a small
collective, and its shard copy rides the (idle-by-then) sync queue.
"""

import numpy as np

import concourse.bass as bass
import concourse.mybir as mybir
import concourse.tile as tile
from concourse import bacc
from concourse.alu_op_type import AluOpType
from concourse.masks import make_identity, make_upper_triangular

F32 = mybir.dt.float32
BF16 = mybir.dt.bfloat16

N_CORES = 8
NHL = 4           # local q heads per core
HD = 128          # head dim
THETA = 10000.0
SCALE = HD ** -0.5
TW = 512          # token block width (free dim of most matmuls)
KW = 128          # key tile width (partition dim of score tiles)

# module-level knobs for test.py
TRACE = False
LAST_RESULTS = None


class Cfg:
    def __init__(self, B=2, L=2048, D=4096):
        self.B, self.L, self.D = B, L, D
        self.BL = B * L
        self.DC = D // 128         # contraction chunks for projections
        self.NB = L // TW          # query blocks per batch
        self.NT = self.BL // TW    # token blocks total
        self.KT = L // KW          # key tiles per batch
        self.NBLK = D // TW        # wo output column blocks
        assert self.BL % TW == 0 and TW % N_CORES == 0
        # ReduceScatter chunks in token order: one full block each, except
        # the LAST PROCESSED block (qb=NB-1 of the last batch), split in
        # half to shrink the tail collective.
        self.chunks = []           # (start_token, width)
        for c in range(self.NT):
            if c == self.NT - 1 and TW >= 2 * N_CORES:
                self.chunks.append((c * TW, TW // 2))
                self.chunks.append((c * TW + TW // 2, TW // 2))
            else:
                self.chunks.append((c * TW, TW))
        self.NCH = len(self.chunks)


# stream_shuffle mask: swap 16-partition halves within each 32-partition quadrant
SWAP16 = [(i + 16) % 32 for i in range(32)]


def _rope_perm():
    """Permutation of head_dim rows: pair i=(16q + r) lives at partitions
    32q+r (x1 = even dim 2i) and 32q+16+r (x2 = odd dim 2i+1)."""
    perm = np.zeros(HD, dtype=np.int64)
    for p in range(HD):
        q, r = divmod(p, 32)
        i = 16 * q + (r % 16)
        perm[p] = 2 * i + (0 if r < 16 else 1)
    return perm


def _rope_tables(cfg):
    """cosT/sinT [128, L] in the permuted-partition layout, sin sign-folded."""
    t = np.arange(cfg.L, dtype=np.float64)
    freqs = THETA ** (-np.arange(0, HD, 2, dtype=np.float64) / HD)  # [64]
    theta = t[None, :] * freqs[:, None]                             # [64, L]
    cos, sin = np.cos(theta), np.sin(theta)
    C = np.zeros((HD, cfg.L), dtype=np.float32)
    S = np.zeros((HD, cfg.L), dtype=np.float32)
    for p in range(HD):
        q, r = divmod(p, 32)
        i = 16 * q + (r % 16)
        C[p] = cos[i]
        S[p] = sin[i] if r >= 16 else -sin[i]
    return C, S


def classify_mask(mask, cfg):
    """cls[kt][qb] = (kind, off): kind in {'Z','N','M'} for tile
    mask[qb*TW:(qb+1)*TW, kt*KW:(kt+1)*KW]; off = count of leading query
    columns in the tile that are fully masked (safe to skip: exp would
    be exactly 0 there). M tiles must match the causal staircase -- the
    device applies them with a constant triangular multiply."""
    cls = [[None] * cfg.NB for _ in range(cfg.KT)]
    for kt in range(cfg.KT):
        for qb in range(cfg.NB):
            t = mask[qb * TW:(qb + 1) * TW, kt * KW:(kt + 1) * KW]
            if np.all(t == 0.0):
                cls[kt][qb] = ('Z', 0)
            elif np.all(t <= -1e8):
                cls[kt][qb] = ('N', 0)
            else:
                qq = np.arange(qb * TW, (qb + 1) * TW)[:, None]
                kk = np.arange(kt * KW, (kt + 1) * KW)[None, :]
                causal = kk <= qq
                assert np.all((t == 0.0) == causal) and \
                    np.all(t[~causal] <= -1e8), \
                    "partial mask tiles must be causal"
                dead_q = np.all(t <= -1e8, axis=1)  # [TW]
                off = 0
                while off < len(dead_q) and dead_q[off]:
                    off += 1
                off = (off // 64) * 64  # keep offsets 64-aligned
                cls[kt][qb] = ('M', off)
    # guard: every query block must attend to at least one key tile
    for qb in range(cfg.NB):
        assert any(cls[kt][qb][0] != 'N' for kt in range(cfg.KT)), \
            "fully-masked query block unsupported"
    return cls


def build_bass(cfg, cls):
    nc = bacc.Bacc("TRN2", target_bir_lowering=False, debug=False,
                   num_devices=N_CORES, num_swdge_queues=4)

    xT_d = nc.dram_tensor("xT", [cfg.D, cfg.BL], BF16, kind="ExternalInput")
    wqT_d = nc.dram_tensor("wqT", [cfg.D, NHL * HD], BF16, kind="ExternalInput")
    wkT_d = nc.dram_tensor("wkT", [cfg.D, HD], BF16, kind="ExternalInput")
    wvT_d = nc.dram_tensor("wvT", [cfg.D, HD], BF16, kind="ExternalInput")
    woT_d = nc.dram_tensor("woT", [NHL * HD, cfg.D], BF16, kind="ExternalInput")
    ropeC_d = nc.dram_tensor("ropeC", [HD, cfg.L], F32, kind="ExternalInput")
    ropeS_d = nc.dram_tensor("ropeS", [HD, cfg.L], F32, kind="ExternalInput")
    out_d = nc.dram_tensor("out", [cfg.BL // N_CORES, cfg.D], BF16,
                           kind="ExternalOutput")

    rg = [list(range(N_CORES))]
    QD = NHL * HD  # 512

    with tile.TileContext(nc) as tc:
        # ---- constants ----------------------------------------------------
        const_pool = tc.alloc_tile_pool(name="const", bufs=1)
        ones_sb = const_pool.tile([128, 128], BF16, name="ones_sb")
        nc.vector.memset(ones_sb[:], 1.0)
        ident = const_pool.tile([128, 128], BF16, name="ident")
        make_identity(nc, ident[:])
        # causal staircase: tri[p, u] = 1 iff u >= p (keep key p on query u)
        tri_sb = const_pool.tile([128, 128], BF16, name="tri_sb")
        make_upper_triangular(nc, tri_sb[:], val=1.0, diag=True)

        # ---- resident activations -----------------------------------------
        kv_pool = tc.alloc_tile_pool(name="kv", bufs=1)
        kT_sb = kv_pool.tile([HD, cfg.BL], BF16, name="kT_sb")
        v_sb = kv_pool.tile([128, cfg.BL], BF16, name="v_sb")
        q_pool = tc.alloc_tile_pool(name="qres", bufs=1)
        qT_sb = q_pool.tile([HD, NHL * cfg.BL], BF16, name="qT_sb")

        # DRAM scratch: wo partials staged per RS chunk
        dram_pool = tc.alloc_tile_pool(name="dram", bufs=1, space="DRAM")
        rs_in = [dram_pool.tile([w, cfg.D], BF16, name=f"rs_in{i}")
                 for i, (s, w) in enumerate(cfg.chunks)]
        rs_out = [dram_pool.tile([w // N_CORES, cfg.D], BF16,
                                 name=f"rs_out{i}")
                  for i, (s, w) in enumerate(cfg.chunks)]

        # ---- weights: wo persists, the rest release after phase 1 ---------
        wo_pool = tc.alloc_tile_pool(name="wo_w", bufs=1)
        wo_sb = wo_pool.tile([128, NHL * cfg.D], BF16, name="wo_sb")
        rtbl_pool = tc.alloc_tile_pool(name="ropetbl", bufs=1)
        ropeC = rtbl_pool.tile([HD, cfg.L], F32, name="ropeC_sb")
        ropeS = rtbl_pool.tile([HD, cfg.L], F32, name="ropeS_sb")
        rtmp_pool = tc.alloc_tile_pool(name="ropetmp", bufs=3)
        vst_pool = tc.alloc_tile_pool(name="vstage", bufs=2)
        x_pool = tc.alloc_tile_pool(name="xload", bufs=8)
        w_pool = tc.alloc_tile_pool(name="weights", bufs=1)
        wq_sb = w_pool.tile([128, cfg.DC * QD], BF16, name="wq_sb")
        wk_sb = w_pool.tile([128, cfg.DC * HD], BF16, name="wk_sb")
        wv_sb = w_pool.tile([128, cfg.DC * HD], BF16, name="wv_sb")

        def load_w3d(eng, dst, src_d, width, chunk, interleave=None):
            """dst[:, dc*width+c] = src[dc*128+p, c], batched `chunk` dcs/DMA.
            With interleave=(dst2, src2): alternate chunks of two tensors."""
            for d0 in range(0, cfg.DC, chunk):
                d1 = min(d0 + chunk, cfg.DC)
                for dd, ss in ((dst, src_d),) + (interleave or ()):
                    eng.dma_start(
                        out=dd[:, d0 * width:d1 * width]
                        .rearrange("p (dc c) -> p dc c", dc=d1 - d0),
                        in_=ss.ap()[d0 * 128:d1 * 128, :]
                        .rearrange("(dc p) c -> p dc c", p=128))

        # wk/wv on the SWDGE queue (gpsimd) so they don't delay x on sync;
        # interleaved so the first dc chunks of BOTH land early.
        load_w3d(nc.gpsimd, wk_sb, wkT_d, HD, 8, interleave=((wv_sb, wvT_d),))
        load_w3d(nc.scalar, wq_sb, wqT_d, QD, 4)   # 8 DMAs of 1MB (ACT queue)
        for h in range(NHL):                       # 4 DMAs of 1MB (ACT queue)
            nc.scalar.dma_start(out=wo_sb[:, h * cfg.D:(h + 1) * cfg.D],
                                in_=woT_d.ap()[h * HD:(h + 1) * HD, :])
        nc.scalar.dma_start(out=ropeC[:], in_=ropeC_d.ap())
        nc.scalar.dma_start(out=ropeS[:], in_=ropeS_d.ap())

        # ================= phase 1: QKV projections + RoPE =================
        with tc.tile_pool(name="qpsum", bufs=1, space="PSUM") as q_psum, \
             tc.tile_pool(name="kpsum", bufs=2, space="PSUM") as k_psum, \
             tc.tile_pool(name="vpsum", bufs=1, space="PSUM") as v_psum, \
             tc.tile_pool(name="vtpsum", bufs=1, space="PSUM") as vt_psum:

            def rope_drain(ps, dst):
                """dst = ps*C + shuffle16(ps)*S (tables sliced at t0)."""
                sw = rtmp_pool.tile([128, TW], F32, name="rope_sw")
                t1 = rtmp_pool.tile([128, TW], F32, name="rope_t1")
                t2 = rtmp_pool.tile([128, TW], F32, name="rope_t2")
                nc.vector.stream_shuffle(sw[:], ps, SWAP16)
                nc.vector.tensor_tensor(t1[:], sw[:], Sx, AluOpType.mult)
                nc.vector.tensor_tensor(t2[:], ps, Cx, AluOpType.mult)
                nc.vector.tensor_tensor(dst, t1[:], t2[:], AluOpType.add)

            for tb in range(cfg.NT):
                t0 = (tb % cfg.NB) * TW  # position within batch
                Cx = ropeC[:, t0:t0 + TW]
                Sx = ropeS[:, t0:t0 + TW]

                q_ps = q_psum.tile([128, NHL * TW], F32, name="q_ps")
                k_ps = k_psum.tile([128, TW], F32, name="k_ps")
                vT_ps = v_psum.tile([128, TW], F32, name="vT_ps")
                xts = []
                XB = 2 if tb == 0 else 4  # dc-chunks per DMA (small first)
                for dc in range(0, cfg.DC, XB):
                    d1 = min(dc + XB, cfg.DC)
                    xt = x_pool.tile([128, (d1 - dc) * TW], BF16, name="x_t")
                    nc.sync.dma_start(
                        out=xt[:].rearrange("p (dc t) -> p dc t", dc=d1 - dc),
                        in_=xT_d.ap()[dc * 128:d1 * 128,
                                      tb * TW:(tb + 1) * TW]
                        .rearrange("(dc p) t -> p dc t", p=128))
                    for j in range(d1 - dc):
                        xts.append(xt[:, j * TW:(j + 1) * TW])
                for dc in range(cfg.DC):
                    st = dict(start=(dc == 0), stop=(dc == cfg.DC - 1))
                    nc.tensor.matmul(k_ps[:],
                                     wk_sb[:, dc * HD:(dc + 1) * HD],
                                     xts[dc], **st)
                    nc.tensor.matmul(vT_ps[:],
                                     wv_sb[:, dc * HD:(dc + 1) * HD],
                                     xts[dc], **st)
                for dc in range(cfg.DC):
                    st = dict(start=(dc == 0), stop=(dc == cfg.DC - 1))
                    for h in range(NHL):
                        nc.tensor.matmul(
                            q_ps[:, h * TW:h * TW + TW],
                            wq_sb[:, dc * QD + h * HD: dc * QD + (h + 1) * HD],
                            xts[dc], **st)

                # k: rope -> resident (drain first: next tb needs this bank)
                rope_drain(k_ps[:], kT_sb[:, tb * TW:(tb + 1) * TW])
                for h in range(NHL):
                    rope_drain(q_ps[:, h * TW:h * TW + TW],
                               qT_sb[:, h * cfg.BL + tb * TW:
                                     h * cfg.BL + (tb + 1) * TW])
                # v: vT -> transpose -> resident [ktok, hd] blocks
                vt_sb = vst_pool.tile([128, TW], BF16, name="vT_stage")
                nc.scalar.copy(vt_sb[:], vT_ps[:])
                for i in range(TW // 128):
                    vp = vt_psum.tile([128, 128], BF16, name="v_tr_ps")
                    nc.tensor.transpose(vp[:], vt_sb[:, i * 128:(i + 1) * 128],
                                        ident[:])
                    nc.scalar.copy(
                        v_sb[:, tb * TW + i * 128: tb * TW + (i + 1) * 128],
                        vp[:])

        w_pool.release()
        x_pool.release()
        vst_pool.release()
        rtmp_pool.release()
        rtbl_pool.release()

        # ================= phase 2: attention + wo + ReduceScatter =========
        with tc.tile_pool(name="expsb", bufs=4) as e_pool, \
             tc.tile_pool(name="attnsb", bufs=2) as at_pool, \
             tc.tile_pool(name="recsb", bufs=2) as rec_pool, \
             tc.tile_pool(name="outcp", bufs=8) as oc_pool, \
             tc.tile_pool(name="scps", bufs=3, space="PSUM") as sc_psum, \
             tc.tile_pool(name="avps", bufs=2, space="PSUM") as av_psum, \
             tc.tile_pool(name="seps", bufs=1, space="PSUM") as se_psum, \
             tc.tile_pool(name="ops", bufs=2, space="PSUM") as o_psum:

            nch_done = 0
            for qb in range(cfg.NB):
                active = [kt for kt in range(cfg.KT)
                          if cls[kt][qb][0] != 'N']
                offs = {kt: cls[kt][qb][1] for kt in active}
                offs[active[0]] = 0

                for b in range(cfg.B):
                    attn_sb = at_pool.tile([128, NHL * TW], BF16, name="at_sb")
                    tb2 = b * cfg.NB + qb
                    for h in range(NHL):
                        qt = qT_sb[:, h * cfg.BL + tb2 * TW:
                                   h * cfg.BL + (tb2 + 1) * TW]
                        at_ps = av_psum.tile([HD, TW], F32, name="at_ps")
                        esum = e_pool.tile([KW, TW], BF16, name="esum",
                                           bufs=2)
                        # software pipeline: issue score matmuls LOOKAHEAD
                        # iterations ahead so the PE never waits on exp (ACT)
                        LOOKAHEAD = 2
                        n_act = len(active)
                        sc_tiles = [None] * n_act

                        def emit_sc(j):
                            kt2 = active[j]
                            gk2 = b * cfg.L + kt2 * KW
                            o = offs[kt2]
                            sc = sc_psum.tile([KW, TW], F32, name="sc_ps")
                            nc.tensor.matmul(sc[:, o:], kT_sb[:, gk2:gk2 + KW],
                                             qt[:, o:], start=True, stop=True)
                            sc_tiles[j] = sc

                        for j in range(min(LOOKAHEAD, n_act)):
                            emit_sc(j)
                        for idx, kt in enumerate(active):
                            if idx + LOOKAHEAD < n_act:
                                emit_sc(idx + LOOKAHEAD)
                            gk = b * cfg.L + kt * KW  # global key token
                            o = offs[kt]
                            sc_ps = sc_tiles[idx]
                            sc_tiles[idx] = None
                            ex = e_pool.tile([KW, TW], BF16, name="ex_t")
                            nc.scalar.activation(
                                ex[:, o:], sc_ps[:, o:],
                                mybir.ActivationFunctionType.Exp,
                                scale=float(SCALE))
                            if cls[kt][qb][0] == 'M':
                                # zero the masked staircase, which for
                                # 128-aligned tiles is exactly the 128
                                # query columns starting at w0: key p is
                                # kept on window column u iff u >= p.
                                w0 = kt * KW - qb * TW
                                nc.vector.tensor_tensor(
                                    ex[:, w0:w0 + KW], ex[:, w0:w0 + KW],
                                    tri_sb[:], AluOpType.mult)
                            # denominator: accumulate exp tiles on the DVE;
                            # one ones-matmul at the end broadcasts the sum
                            if idx == 0:
                                nc.vector.tensor_copy(esum[:], ex[:])
                            else:
                                nc.vector.tensor_tensor(
                                    esum[:, o:], esum[:, o:], ex[:, o:],
                                    AluOpType.add)
                            st = dict(start=(idx == 0),
                                      stop=(idx == len(active) - 1))
                            nc.tensor.matmul(at_ps[:, o:], v_sb[:, gk:gk + KW],
                                             ex[:, o:], **st)
                        se_ps = se_psum.tile([128, TW], F32, name="se_ps")
                        nc.tensor.matmul(se_ps[:], ones_sb[:], esum[:],
                                         start=True, stop=True)
                        rec = rec_pool.tile([128, TW], F32, name="rec_t")
                        nc.vector.reciprocal_approx_fast(rec[:], se_ps[:])
                        nc.vector.tensor_tensor(
                            attn_sb[:, h * TW:(h + 1) * TW],
                            at_ps[:], rec[:], AluOpType.mult)

                    # ---- wo partial for this (b, qb) token block ----------
                    c = b * cfg.NB + qb
                    base_tok = c * TW
                    NG = min(4, cfg.NBLK)  # n-blocks per batched store
                    block_chunks = [i for i, (s, w) in enumerate(cfg.chunks)
                                    if base_tok <= s < base_tok + TW]
                    for m in range(TW // 128):
                        tok = base_tok + m * 128
                        c2 = next(i for i, (s, w) in enumerate(cfg.chunks)
                                  if s <= tok < s + w)
                        mr = tok - cfg.chunks[c2][0]
                        for ng in range(cfg.NBLK // NG):
                            oc = oc_pool.tile([128, NG * TW], BF16,
                                              name="oc_t")
                            for j in range(NG):
                                n = ng * NG + j
                                o_ps = o_psum.tile([128, TW], F32,
                                                   name="o_ps")
                                for h in range(NHL):
                                    nc.tensor.matmul(
                                        o_ps[:],
                                        attn_sb[:, h * TW + m * 128:
                                                h * TW + (m + 1) * 128],
                                        wo_sb[:, h * cfg.D + n * TW:
                                              h * cfg.D + (n + 1) * TW],
                                        start=(h == 0), stop=(h == NHL - 1))
                                # alternate drain engine: keep ACT free
                                dst = oc[:, j * TW:(j + 1) * TW]
                                if n % 2 == 0:
                                    nc.vector.tensor_copy(dst, o_ps[:])
                                else:
                                    nc.scalar.copy(dst, o_ps[:])
                            # stage to DRAM on the sync HWDGE queue: the
                            # collective's queue must never gate compute
                            nc.sync.dma_start(
                                out=rs_in[c2][mr:mr + 128,
                                              ng * NG * TW:(ng + 1) * NG * TW],
                                in_=oc[:])
                    # ReduceScatter each finished chunk; the gpsimd stream
                    # holds only collectives + shard copies
                    for c2 in block_chunks:
                        s, w = cfg.chunks[c2]
                        rw = w // N_CORES
                        ro = s // N_CORES  # output row offset
                        nc.gpsimd.collective_compute(
                            "ReduceScatter", AluOpType.add, replica_groups=rg,
                            ins=[rs_in[c2][:].opt()],
                            outs=[rs_out[c2][:].opt()])
                        nch_done += 1
                        cp_eng = nc.sync if nch_done == cfg.NCH else nc.gpsimd
                        cp_eng.dma_start(
                            out=out_d.ap()[ro:ro + rw, :],
                            in_=rs_out[c2][:])

        dram_pool.release()
        wo_pool.release()
        q_pool.release()
        kv_pool.release()
        const_pool.release()

    nc.compile()
    return nc


def host_prepare(cfg, x, mask, wq, wk, wv, wo):
    """Returns (in_maps, cls)."""
    x = np.ascontiguousarray(np.asarray(x, dtype=np.float32))
    mask = np.asarray(mask, dtype=np.float32)
    wq = np.asarray(wq, dtype=np.float32)
    wk = np.asarray(wk, dtype=np.float32)
    wv = np.asarray(wv, dtype=np.float32)
    wo = np.asarray(wo, dtype=np.float32)

    import ml_dtypes
    bf16 = ml_dtypes.bfloat16
    perm = _rope_perm()
    C, S = _rope_tables(cfg)
    xT = np.ascontiguousarray(x.reshape(cfg.BL, cfg.D).T).astype(bf16)
    cls = classify_mask(mask, cfg)

    in_maps = []
    for g in range(N_CORES):
        qrows = wq[g * NHL * HD:(g + 1) * NHL * HD]          # [512, D]
        qperm = np.concatenate(
            [qrows[h * HD + perm] for h in range(NHL)], axis=0)
        krows = wk[g * HD:(g + 1) * HD][perm]                # [128, D]
        vrows = wv[g * HD:(g + 1) * HD]                      # [128, D]
        wocols = wo[:, g * NHL * HD:(g + 1) * NHL * HD]      # [D, 512]
        in_maps.append({
            "xT": xT,
            "wqT": np.ascontiguousarray(qperm.T).astype(bf16),
            "wkT": np.ascontiguousarray(krows.T).astype(bf16),
            "wvT": np.ascontiguousarray(vrows.T).astype(bf16),
            "woT": np.ascontiguousarray(wocols.T).astype(bf16),
            "ropeC": C,
            "ropeS": S,
        })
    return in_maps, cls


def assemble_output(cfg, results):
    """Stitch per-core ReduceScatter shards back into [B, L, D]."""
    full = np.empty((cfg.BL, cfg.D), dtype=np.float32)
    for g in range(N_CORES):
        r = np.asarray(results[g]["out"]).astype(np.float32)
        for s, w in cfg.chunks:
            rw = w // N_CORES
            ro = s // N_CORES
            full[s + g * rw: s + (g + 1) * rw] = r[ro:ro + rw]
    return full.reshape(cfg.B, cfg.L, cfg.D)


def kernel(x, mask, wq, wk, wv, wo):
    global LAST_RESULTS
    from concourse.bass_utils import run_bass_kernel_spmd
    cfg = Cfg(B=2, L=2048, D=4096)
    in_maps, cls = host_prepare(cfg, x, mask, wq, wk, wv, wo)
    nc = build_bass(cfg, cls)
    res = run_bass_kernel_spmd(nc, in_maps, core_ids=list(range(N_CORES)),
                               trace=TRACE)
    LAST_RESULTS = res
    return assemble_output(cfg, res.results)
